# revision 67
# baseline (speedup 1.0000x reference)
"""Trainium2 Bass kernel for the EntangledInterferenceLayer problem.

Math transformations done on host (numpy), all exact up to fp rounding:
  * The HxH entanglement mix commutes with RoPE (cos/sin are head-independent),
    so it folds into the Q/K projection weights + biases.
  * The per-head phase shift rotates q and k by the same complex phase, and the
    attention logits use q * conj(k) -> the phase cancels exactly.  Dropped.
  * 1/sqrt(head_dim) folds into the Q weights/bias.
  * The V-projection bias contributes bv @ Wo to every output row (softmax rows
    sum to 1), so it folds into the output bias.

Sharding (8 cores): core = (batch b, head-group g of 4 heads). Each core
projects Q/K/V for its heads, runs causal complex-magnitude attention, then an
AllGather (bf16) of attention outputs within the 4-core batch group lets every
core compute a 256-column slice of both output projections.

Device layout notes:
  * All weights / x / rope tables are pre-arranged on the host into the exact
    SBUF layout ([128 partitions, kt, cols]) so every load is one contiguous
    DMA with large descriptors.
  * Q/K are computed transposed ([head-dim, token]); per head the 128
    contraction rows are [qr-rot, qr-nr, qi-rot, qi-nr]; K2 = [-ki, kr] gives
    the imaginary logits with plain matmuls.
  * Scores are built [kv, q]; causality is exploited inside tiles by slicing
    all elementwise work (and matmuls where free>=256 keeps fp32r full rate)
    to the unmasked q-range; only diagonal tiles get a 128-wide affine_select.
  * sqrt/exp batched per head-pair so ACT table loads are amortized.
  * Softmax denominators accumulate in a dedicated PSUM row via ones-matmuls;
    one reciprocal per head-pair, broadcast across partitions with an
    indicator matmul; normalization multiplies read the AV sums directly from
    PSUM (no evacuation copies).
"""

import math

import numpy as np

B, S, DIM = 2, 1024, 1024
HEADS, HD, ROTD = 16, 64, 32
GH = 4  # heads per core
ODC = 256  # out-dim columns per core
NCORES = 8

_PAIRSWAP = [i ^ 1 for i in range(32)]

MM_DTYPE = "float32r"


def _register_magsq():
    """Register a fused custom DVE op: out = (in0^2 + in1^2) * imm2."""
    import numpy as np
    from concourse import dve_ops as DO
    from concourse.dve_spec import Spec, Src0, Src1, C2, sq, lower

    if "ANT_MAGSQ" in DO._SUB_OPCODE_FOR_NAME:
        return next(o for o in DO.OPS if o.name == "ANT_MAGSQ")
    spec = Spec(
        body=(sq(Src0) + sq(Src1)) * C2,
        reference=lambda in0, in1, s0, s1, imm2: (
            in0.astype(np.float32) ** 2 + in1.astype(np.float32) ** 2
        )
        * np.float32(imm2),
    )
    opcode = DO._CUSTOM_DVE_ROW_BASE + len(DO.OPS)
    DO._SUB_OPCODE_FOR_NAME["ANT_MAGSQ"] = opcode
    shas = {}
    for ver in ("v3", "v4"):
        try:
            s = DO.DveOpSpec(
                name="ANT_MAGSQ", opcode=opcode, uops=lower(spec, ver=ver), rd1_en=True
            )
            shas[ver] = s.sha(ver)
        except Exception:
            pass
    op = DO.DveOp("ANT_MAGSQ", spec, subdim=False, uops_sha=shas)
    DO.OPS.append(op)
    DO.CUSTOM_DVE_SPECS["ANT_MAGSQ"] = spec
    return op


def _build(gt: float, groups=None, mm_dtype=None):
    import concourse.mybir as mybir
    import concourse.tile as tile
    from concourse import bacc

    f32 = mybir.dt.float32
    bf16 = mybir.dt.bfloat16
    mdt = getattr(mybir.dt, mm_dtype or MM_DTYPE)
    AF = mybir.ActivationFunctionType
    magsq = _register_magsq()
    gt2 = float(gt) * float(gt)

    nc = bacc.Bacc("TRN2", target_bir_lowering=False, num_devices=NCORES)
    if groups is None:
        groups = [[0, 1, 2, 3], [4, 5, 6, 7]]

    # bf16 inputs halve the load-phase DMA (the kernel-start critical path)
    xr = nc.dram_tensor("xr_t", [128, 8, S], bf16, kind="ExternalInput")
    xi = nc.dram_tensor("xi_t", [128, 8, S], bf16, kind="ExternalInput")
    w = {
        nm: nc.dram_tensor(nm, [128, 8, 256], bf16, kind="ExternalInput")
        for nm in ["wqr", "wqi", "wkr", "wki", "wvr", "wvi"]
    }
    for nm in ["wor", "woi"]:
        w[nm] = nc.dram_tensor(nm, [128, 8, 256], bf16, kind="ExternalInput")
    bias_d = {
        nm: nc.dram_tensor(nm, [1, 256], mdt, kind="ExternalInput")
        for nm in ["bqr", "bqi", "bkr", "bki"]
    }
    # per-partition bias columns folded into PSUM-evacuation activations
    bnr_d = nc.dram_tensor("bnr", [128, 4], f32, kind="ExternalInput")
    boc_d = nc.dram_tensor("boc", [128, 4], f32, kind="ExternalInput")
    cosd = nc.dram_tensor("cosd", [128, S], f32, kind="ExternalInput")
    sind = nc.dram_tensor("sind", [128, S], f32, kind="ExternalInput")
    o_r = nc.dram_tensor("o_r", [ODC, S], bf16, kind="ExternalOutput")
    o_i = nc.dram_tensor("o_i", [ODC, S], bf16, kind="ExternalOutput")
    # per-(chunk, head-pair) gather buffers: 4 small collectives pipeline the
    # communication behind compute; block rows = [h0r h1r h0i h1i]
    agin_d = [
        nc.dram_tensor(f"agin{qc}", [512, 512], bf16, kind="Internal")
        for qc in range(2)
    ]
    agout_d = [
        nc.dram_tensor(f"agout{qc}{hp}", [4, 256, 512], bf16, kind="Internal")
        for qc in range(2)
        for hp in range(2)
    ]

    def mm(out, lhsT, rhs, start, stop):
        nc.tensor.matmul(out, lhsT=lhsT, rhs=rhs, start=start, stop=stop)

    # Rotate staging DMAs over the DMA-capable sequencers (SP/ACT HWDGE plus
    # the otherwise-idle gpsimd SWDGE).
    class DmaRotor:
        def __init__(self, engines):
            self.engines = engines
            self.i = 0

        def __call__(self):
            e = self.engines[self.i % len(self.engines)]
            self.i += 1
            return e

    rot = DmaRotor([nc.sync, nc.scalar, nc.sync, nc.gpsimd])
    rot2 = DmaRotor([nc.sync, nc.scalar])

    with tile.TileContext(nc) as tc:
        with (
            tc.tile_pool(name="consts", bufs=1) as consts,
            tc.tile_pool(name="qkpersist", bufs=1) as qkp,
            tc.tile_pool(name="vpersist", bufs=1) as vper,
            tc.tile_pool(name="wop", bufs=1) as wop,
            tc.tile_pool(name="ps", bufs=7, space="PSUM") as ps,
            tc.tile_pool(name="psd", bufs=1, space="PSUM") as psd,
        ):
            ones_t = consts.tile([1, 512], mdt, tag="ones")
            nc.vector.memset(ones_t.bitcast(f32), 1.0)
            # ones rows at partition bases 0 and 32 (matmul requires lhsT and
            # rhs to share their base partition) for the reciprocal broadcast
            ones_row = consts.tile([33, 64], f32, tag="onesr")
            nc.vector.memset(ones_row, 1.0)
            eps_t = consts.tile([128, 1], f32, tag="eps")
            nc.vector.memset(eps_t, 1e-6 * gt2)
            # bias tiles allocated here; DMAs issued after the first weight/x
            # loads so they don't delay the first matmul's inputs
            bias_sb = {
                nm: consts.tile([1, 256], mdt, tag=nm, name=f"bias_{nm}")
                for nm in ["bqr", "bqi", "bkr", "bki"]
            }
            bnr_sb = consts.tile([128, 4], f32, tag="bnr")
            boc_sb = consts.tile([128, 4], f32, tag="boc")

            # bf16 Q/K halves staging traffic, SBUF and stationary-load time;
            # logits stay fp32 in PSUM (measured rel err stays well in spec)
            Q = qkp.tile([128, GH, S], bf16, tag="Q")
            K1 = qkp.tile([128, GH, S], bf16, tag="K1")
            K2 = qkp.tile([128, GH, S], bf16, tag="K2")
            ovr = vper.tile([128, 8, 256], bf16, tag="ovr")
            ovi = vper.tile([128, 8, 256], bf16, tag="ovi")
            onescol_bf = consts.tile([128, 1], bf16, tag="onesc")
            nc.vector.memset(onescol_bf, 1.0)
            # wo tiles allocated here; their DMAs are issued after the
            # projection inputs (they're only needed at the very end)
            wo_sb = {}
            for ri, wname in ((0, "wor"), (1, "woi")):
                wo_sb[ri] = wop.tile([128, 8, 256], bf16, tag=wname, name=f"wo{ri}")

            # ---- projection phase (scoped pools so SBUF frees afterwards) ----
            with (
                tc.tile_pool(name="xp", bufs=1) as xp,
                tc.tile_pool(name="wqkp", bufs=1) as wqkp,
                tc.tile_pool(name="ropep", bufs=1) as ropep,
                tc.tile_pool(name="stage", bufs=2) as stage,
                tc.tile_pool(name="evp", bufs=2) as evp,
            ):
                # input DMAs in need-order: descriptors drain roughly in issue
                # order across the shared queues, so the first projection can
                # begin within a few microseconds
                w_sb = {}
                x_sb = {"r": [], "i": []}

                def load_w(nm):
                    t = wqkp.tile([128, 8, 256], bf16, tag=f"w{nm}")
                    rot2().dma_start(t, w[nm][:, :, :])
                    w_sb[nm] = t

                def load_x(key, src, kts):
                    for kt in kts:
                        t = xp.tile([128, S], bf16, tag=f"x{key}{kt}")
                        rot2().dma_start(t, src[:, kt, :])
                        x_sb[key].append(t)

                load_w("wqr")
                load_x("r", xr, range(8))
                for nm in ["bqr", "bqi", "bkr", "bki"]:
                    nc.sync.dma_start(bias_sb[nm], bias_d[nm][:, :])
                nc.scalar.dma_start(bnr_sb, bnr_d[:, :])
                nc.scalar.dma_start(boc_sb, boc_d[:, :])
                cos_sb = ropep.tile([128, S], f32, tag="cos")
                nc.sync.dma_start(cos_sb, cosd[:, :])
                sin_sb = ropep.tile([128, S], f32, tag="sin")
                nc.scalar.dma_start(sin_sb, sind[:, :])
                load_w("wqi")
                load_x("i", xi, range(8))
                for nm in ("wkr", "wki", "wvr", "wvi"):
                    load_w(nm)
                for ri, wname in ((0, "wor"), (1, "woi")):
                    rot2().dma_start(wo_sb[ri], w[wname][:, :, :])

                projs = [
                    ("qr", "r", "wqr", "bqr", [(0, 0)], [(0, 32)]),
                    ("qi", "i", "wqi", "bqi", [(0, 64)], [(0, 96)]),
                    ("kr", "r", "wkr", "bkr", [(1, 0), (2, 64)], [(1, 32), (2, 96)]),
                    ("ki", "i", "wki", "bki", [(1, 64)], [(1, 96)]),
                ]
                qk_tensors = {0: Q, 1: K1, 2: K2}

                for pidx, (pname, xkey, wname, bname, rot_tgts, nr_tgts) in enumerate(
                    projs
                ):
                    wt = w_sb[wname]
                    for mt in range(2):
                        psts = [
                            ps.tile([128, 512], f32, tag="ps", name=f"pq{c}")
                            for c in range(2)
                        ]
                        for kt in range(8):
                            for c in range(2):
                                mm(
                                    psts[c],
                                    wt[:, kt, mt * 128 : (mt + 1) * 128],
                                    x_sb[xkey][kt][:, c * 512 : (c + 1) * 512],
                                    start=(kt == 0),
                                    stop=(kt == 7 and mt == 1),
                                )
                        if mt == 0:
                            # rot-dim bias must be added before the rotation
                            for c in range(2):
                                mm(
                                    psts[c],
                                    bias_sb[bname][:, 0:128],
                                    ones_t,
                                    start=False,
                                    stop=True,
                                )
                        for c in range(2):
                            csl = slice(c * 512, (c + 1) * 512)
                            pst = psts[c]
                            if mt == 0:
                                shuf = stage.tile([128, 512], f32, tag="shuf")
                                nc.vector.stream_shuffle(shuf, pst, mask=_PAIRSWAP)
                                nc.vector.tensor_mul(shuf, shuf, sin_sb[:, csl])
                                t2f = stage.tile([128, 512], f32, tag="t2f")
                                nc.vector.tensor_mul(t2f, pst, cos_sb[:, csl])
                                t2 = stage.tile([128, 512], bf16, tag="t2")
                                nc.vector.tensor_add(t2, t2f, shuf)
                                src_t = t2
                            else:
                                # non-rot bias folds into the evacuation
                                evn = evp.tile([128, 512], bf16, tag="ev")
                                nc.scalar.activation(
                                    evn,
                                    pst,
                                    AF.Identity,
                                    bias=bnr_sb[:, pidx : pidx + 1],
                                )
                                src_t = evn
                            tgts = rot_tgts if mt == 0 else nr_tgts
                            for tid, row0 in tgts:
                                dst = qk_tensors[tid]
                                for h in range(GH):
                                    rot().dma_start(
                                        dst[row0 : row0 + 32, h, csl],
                                        src_t[h * 32 : (h + 1) * 32, :],
                                    )
                            if pname == "ki":
                                neg = evp.tile([128, 512], bf16, tag="neg")
                                nc.vector.tensor_scalar_mul(neg, src_t, -1.0)
                                row0 = 0 if mt == 0 else 32
                                for h in range(GH):
                                    rot().dma_start(
                                        K2[row0 : row0 + 32, h, csl],
                                        neg[h * 32 : (h + 1) * 32, :],
                                    )

                # V projections -> directly into persistent [tok, (h d)] tiles
                for xkey, wvname, ovt in (("r", "wvr", ovr), ("i", "wvi", ovi)):
                    wv_sb = w_sb[wvname]
                    for tt in range(8):
                        pv = ps.tile([128, 256], f32, tag="ps", name="pv")
                        for kt in range(8):
                            mm(
                                pv,
                                x_sb[xkey][kt][:, tt * 128 : (tt + 1) * 128],
                                wv_sb[:, kt, :],
                                start=(kt == 0),
                                stop=(kt == 7),
                            )
                        nc.scalar.copy(ovt[:, tt, :], pv)

            # ---- attention ----
            with (
                tc.tile_pool(name="sqp", bufs=40) as sqp,
                tc.tile_pool(name="etp", bufs=34) as etp,
                tc.tile_pool(name="ccp", bufs=4) as ccp,
                tc.tile_pool(name="onp", bufs=4) as onp,
                tc.tile_pool(name="smallp", bufs=2) as smallp,
                tc.tile_pool(name="lop", bufs=4) as lop,
                tc.tile_pool(name="oop", bufs=2) as oop,
            ):
                def score_phase(qc, act_copies):
                    """Scores + |z|^2 for all heads of chunk qc."""
                    nkv = (qc + 1) * 4
                    qcs = slice(qc * 512, (qc + 1) * 512)
                    sq_list = []
                    for h in range(GH):
                        qsl = Q[:, h, qcs]
                        for kvt in range(nkv):
                            loc = max(0, kvt * 128 - qc * 512)
                            ksl = slice(kvt * 128, (kvt + 1) * 128)
                            psr = ps.tile([128, 512], f32, tag="ps", name="psr")
                            mm(
                                psr[:, loc:],
                                K1[:, h, ksl],
                                qsl[:, loc:],
                                start=True,
                                stop=True,
                            )
                            psi = ps.tile([128, 512], f32, tag="ps", name="psi")
                            mm(
                                psi[:, loc:],
                                K2[:, h, ksl],
                                qsl[:, loc:],
                                start=True,
                                stop=True,
                            )
                            # GPSIMD can't read PSUM; the psi copy goes to ACT
                            # only when ACT isn't busy with another chunk's
                            # sqrt/exp batches.
                            c1 = ccp.tile([128, 512], f32, tag="c1")
                            if act_copies and (h + kvt) % 2 == 0:
                                nc.scalar.copy(c1[:, loc:], psi[:, loc:])
                            else:
                                nc.vector.tensor_copy(c1[:, loc:], psi[:, loc:])
                            sq = sqp.tile([128, 512], f32, tag="sq")
                            nc.vector._custom_dve(
                                magsq,
                                out=sq[:, loc:],
                                in0=psr[:, loc:],
                                in1=c1[:, loc:],
                                imm2=gt2,
                            )
                            sq_list.append((h, kvt, loc, sq))
                    return sq_list

                def act_phase(qc, sq_list):
                    """Batched sqrt/exp (amortized table loads) + causal masks."""
                    nkv = (qc + 1) * 4
                    for h, kvt, loc, sq in sq_list:
                        nc.scalar.activation(
                            sq[:, loc:], sq[:, loc:], AF.Sqrt, bias=eps_t
                        )
                    et_map = {}
                    for h, kvt, loc, sq in sq_list:
                        et = etp.tile([128, 512], bf16, tag="et")
                        nc.scalar.activation(
                            et[:, loc:], sq[:, loc:], AF.Exp, scale=1.0
                        )
                        et_map[(h, kvt)] = (et, loc)
                    for h in range(GH):
                        for kvt in range(nkv):
                            off = kvt - qc * 4
                            if off >= 0:
                                et, loc = et_map[(h, kvt)]
                                dlo = off * 128
                                nc.gpsimd.affine_select(
                                    out=et[:, dlo : dlo + 128],
                                    in_=et[:, dlo : dlo + 128],
                                    compare_op=mybir.AluOpType.is_ge,
                                    fill=0.0,
                                    base=0,
                                    channel_multiplier=-1,
                                    pattern=[[1, 128]],
                                )
                    return et_map

                def pair_phase(qc, et_map, hp):
                    nkv = (qc + 1) * 4
                    pair = (2 * hp, 2 * hp + 1)
                    # softmax denominators for the pair: PSUM rows 0 / 32
                    # (matmul out base partition must be 0, 32, or 64)
                    ddd = psd.tile([64, 512], f32, tag="ddd")
                    for hh, h in enumerate(pair):
                        r0 = hh * 32
                        for kvt in range(nkv):
                            et, loc = et_map[(h, kvt)]
                            mm(
                                ddd[r0 : r0 + 1, loc:],
                                onescol_bf,
                                et[:, loc:],
                                start=(kvt == 0),
                                stop=(kvt == nkv - 1),
                            )
                    # AV accumulation
                    avs = {}
                    for h in pair:
                        avr = ps.tile([64, 512], f32, tag="ps", name="avr")
                        avi = ps.tile([64, 512], f32, tag="ps", name="avi")
                        for kvt in range(nkv):
                            et, loc = et_map[(h, kvt)]
                            mm(
                                avr[:, loc:],
                                ovr[:, kvt, h * 64 : (h + 1) * 64],
                                et[:, loc:],
                                start=(kvt == 0),
                                stop=(kvt == nkv - 1),
                            )
                            mm(
                                avi[:, loc:],
                                ovi[:, kvt, h * 64 : (h + 1) * 64],
                                et[:, loc:],
                                start=(kvt == 0),
                                stop=(kvt == nkv - 1),
                            )
                        avs[h] = (avr, avi)
                    # normalize straight out of PSUM and stage for AllGather
                    # (approx reciprocal: denominators are sums of exp >= 1;
                    # garbage partitions 1-31/33-63 are never read)
                    lrec = smallp.tile([64, 512], f32, tag="lrec")
                    nc.vector.reciprocal_approx_fast(out=lrec, in_=ddd[:, :])
                    pb = ps.tile([128, 512], f32, tag="ps", name="pb")
                    mm(pb[0:64, :], ones_row[0:1, :], lrec[0:1, :],
                       start=True, stop=True)
                    mm(pb[64:128, :], ones_row[32:33, :], lrec[32:33, :],
                       start=True, stop=True)
                    pb_sb = smallp.tile([128, 512], f32, tag="pbs")
                    nc.scalar.copy(pb_sb, pb)
                    b0 = hp * 256
                    for hh, h in enumerate(pair):
                        avr, avi = avs[h]
                        bsl = slice(hh * 64, hh * 64 + 64)
                        onn = onp.tile([128, 512], bf16, tag="onn")
                        nc.vector.tensor_mul(onn[0:64, :], avr, pb_sb[bsl, :])
                        nc.vector.tensor_mul(onn[64:128, :], avi, pb_sb[bsl, :])
                        rot2().dma_start(
                            agin_d[qc][b0 + hh * 64 : b0 + (hh + 1) * 64, :],
                            onn[0:64, :],
                        )
                        rot2().dma_start(
                            agin_d[qc][b0 + 128 + hh * 64 : b0 + 128 + (hh + 1) * 64, :],
                            onn[64:128, :],
                        )

                def gather(qc, hp):
                    nc.gpsimd.collective_compute(
                        "AllGather",
                        mybir.AluOpType.bypass,
                        replica_groups=groups,
                        ins=[agin_d[qc][hp * 256 : (hp + 1) * 256, :].opt()],
                        outs=[agout_d[qc * 2 + hp][:, :, :].opt()],
                    )

                for qc in range(2):
                    sq = score_phase(qc, act_copies=True)
                    et = act_phase(qc, sq)
                    pair_phase(qc, et, 0)
                    gather(qc, 0)
                    pair_phase(qc, et, 1)
                    gather(qc, 1)
                for qc in range(2):
                    qcs = slice(qc * 512, (qc + 1) * 512)
                    for ri, bname, odst in ((0, "bor", o_r), (1, "boi", o_i)):
                        pos = [
                            ps.tile([128, 512], f32, tag="ps", name=f"po{_i}")
                            for _i in range(2)
                        ]
                        # hp-outer so the first gather's data is consumed
                        # while the second is still in flight
                        for hp in range(2):
                            for g in range(4):
                                lt = lop.tile([128, 512], bf16, tag="lt")
                                nc.scalar.dma_start(
                                    lt,
                                    agout_d[qc * 2 + hp][
                                        g, ri * 128 : (ri + 1) * 128, :
                                    ],
                                )
                                ht = g * 2 + hp
                                for odt in range(2):
                                    mm(
                                        pos[odt],
                                        wo_sb[ri][:, ht, odt * 128 : (odt + 1) * 128],
                                        lt,
                                        start=(hp == 0 and g == 0),
                                        stop=(hp == 1 and g == 3),
                                    )
                        for odt in range(2):
                            # output bias folds into the evacuation
                            oo = oop.tile([128, 512], bf16, tag="oo")
                            nc.scalar.activation(
                                oo,
                                pos[odt],
                                AF.Identity,
                                bias=boc_sb[:, ri * 2 + odt : ri * 2 + odt + 1],
                            )
                            nc.sync.dma_start(
                                odst[odt * 128 : (odt + 1) * 128, qcs], oo
                            )

    return nc


def _host_prep(inputs):
    """Fold ent/scale/bv on host; build per-core input maps (device layouts)."""
    import ml_dtypes

    bf16 = ml_dtypes.bfloat16
    f = lambda x: np.asarray(x, dtype=np.float32)
    real, imag = f(inputs["real"]), f(inputs["imag"])
    ent = np.asarray(inputs["ent"], np.float64)
    scale = 1.0 / math.sqrt(HD)

    def fold_w(W, do_ent, sc=1.0):
        W = np.asarray(W, np.float64).reshape(DIM, HEADS, HD)
        if do_ent:
            W = np.einsum("chd,hx->cxd", W, ent)
        return W * sc

    def fold_b(b, do_ent, sc=1.0):
        b = np.asarray(b, np.float64).reshape(HEADS, HD)
        if do_ent:
            b = np.einsum("hd,hx->xd", b, ent)
        return b * sc

    Wq_r = fold_w(inputs["Wq_r"], True, scale)
    Wq_i = fold_w(inputs["Wq_i"], True, scale)
    Wk_r = fold_w(inputs["Wk_r"], True)
    Wk_i = fold_w(inputs["Wk_i"], True)
    Wv_r = fold_w(inputs["Wv_r"], False)
    Wv_i = fold_w(inputs["Wv_i"], False)
    bq_r = fold_b(inputs["bq_r"], True, scale)
    bq_i = fold_b(inputs["bq_i"], True, scale)
    bk_r = fold_b(inputs["bk_r"], True)
    bk_i = fold_b(inputs["bk_i"], True)
    Wo_r = np.asarray(inputs["Wo_r"], np.float64)
    Wo_i = np.asarray(inputs["Wo_i"], np.float64)
    bo_r = np.asarray(inputs["bo_r"], np.float64) + np.asarray(
        inputs["bv_r"], np.float64
    ) @ Wo_r
    bo_i = np.asarray(inputs["bo_i"], np.float64) + np.asarray(
        inputs["bv_i"], np.float64
    ) @ Wo_i

    strength = float(np.asarray(inputs["strength"]).reshape(-1)[0])
    temp = float(np.asarray(inputs["temp"]).reshape(-1)[0])
    gt = (1.0 / (1.0 + math.exp(-strength))) / max(temp, 0.01)

    # rope tables in device layout: row h*32+d (d<32), freq j=d//2
    rot_freqs = np.asarray(inputs["rot_freqs"], np.float64)  # [16]
    pos = np.arange(S, dtype=np.float64)
    emb = pos[:, None] * rot_freqs[None, :]  # [S, 16]
    cos_t = np.cos(emb)
    sin_t = np.sin(emb)
    cosd = np.empty((128, S), np.float32)
    sind = np.empty((128, S), np.float32)
    for hh in range(4):
        for d in range(32):
            r = hh * 32 + d
            cosd[r] = cos_t[:, d // 2]
            sind[r] = (-sin_t if d % 2 == 0 else sin_t)[:, d // 2]

    def dev128(a2d, dtype=np.float32):
        # [1024, M] -> [128, 8, M] with row d = kt*128+kp at [kp, kt]
        m = a2d.shape[1]
        return np.ascontiguousarray(
            a2d.reshape(8, 128, m).transpose(1, 0, 2)
        ).astype(dtype)

    def qk_dev(Wf, bf_, g):
        hs = slice(g * GH, (g + 1) * GH)
        Wc, bc = Wf[:, hs, :], bf_[hs, :]
        wd = np.concatenate(
            [
                Wc[:, :, :ROTD].reshape(DIM, GH * ROTD),
                Wc[:, :, ROTD:].reshape(DIM, GH * ROTD),
            ],
            axis=1,
        )
        bd = np.concatenate(
            [bc[:, :ROTD].reshape(1, GH * ROTD), bc[:, ROTD:].reshape(1, GH * ROTD)],
            axis=1,
        )
        return wd, bd

    in_maps = []
    for core in range(NCORES):
        b, g = core // 4, core % 4
        hs = slice(g * GH, (g + 1) * GH)
        m = {
            "xr_t": dev128(np.ascontiguousarray(real[b].T), bf16),
            "xi_t": dev128(np.ascontiguousarray(imag[b].T), bf16),
            "cosd": cosd,
            "sind": sind,
            "wvr": dev128(Wv_r[:, hs, :].reshape(DIM, 256), bf16),
            "wvi": dev128(Wv_i[:, hs, :].reshape(DIM, 256), bf16),
            "wor": dev128(Wo_r[:, g * ODC : (g + 1) * ODC], bf16),
            "woi": dev128(Wo_i[:, g * ODC : (g + 1) * ODC], bf16),
            "boc": np.stack(
                [
                    bo[g * ODC + odt * 128 : g * ODC + (odt + 1) * 128]
                    for bo in (bo_r, bo_i)
                    for odt in range(2)
                ],
                axis=1,
            ).astype(np.float32),
        }
        bnr_cols = []
        for nm, Wf, bf_ in (
            ("qr", Wq_r, bq_r),
            ("qi", Wq_i, bq_i),
            ("kr", Wk_r, bk_r),
            ("ki", Wk_i, bk_i),
        ):
            wd, bd = qk_dev(Wf, bf_, g)
            m["w" + nm] = dev128(wd, bf16)
            m["b" + nm] = bd.astype(np.float32)
            bnr_cols.append(bd[0, 128:256])
        m["bnr"] = np.stack(bnr_cols, axis=1).astype(np.float32)
        in_maps.append(m)
    return in_maps, gt


def kernel(**inputs):
    from concourse import bass_utils

    in_maps, gt = _host_prep(inputs)
    nc = _build(gt)
    nc.finalize()
    res = bass_utils.run_bass_kernel_spmd(
        nc, in_maps, core_ids=list(range(NCORES))
    )
    out_r = np.empty((B, S, DIM), np.float32)
    out_i = np.empty((B, S, DIM), np.float32)
    for core in range(NCORES):
        b, g = core // 4, core % 4
        out_r[b, :, g * ODC : (g + 1) * ODC] = (
            res.results[core]["o_r"].astype(np.float32).T
        )
        out_i[b, :, g * ODC : (g + 1) * ODC] = (
            res.results[core]["o_i"].astype(np.float32).T
        )
    return np.stack([out_r, out_i], axis=0)


# revision 70
# speedup vs baseline: 1.0125x; 1.0125x over previous
"""Trainium2 Bass kernel for the EntangledInterferenceLayer problem.

Math transformations done on host (numpy), all exact up to fp rounding:
  * The HxH entanglement mix commutes with RoPE (cos/sin are head-independent),
    so it folds into the Q/K projection weights + biases.
  * The per-head phase shift rotates q and k by the same complex phase, and the
    attention logits use q * conj(k) -> the phase cancels exactly.  Dropped.
  * 1/sqrt(head_dim) folds into the Q weights/bias.
  * The V-projection bias contributes bv @ Wo to every output row (softmax rows
    sum to 1), so it folds into the output bias.

Sharding (8 cores): core = (batch b, head-group g of 4 heads). Each core
projects Q/K/V for its heads, runs causal complex-magnitude attention, then an
AllGather (bf16) of attention outputs within the 4-core batch group lets every
core compute a 256-column slice of both output projections.

Device layout notes:
  * All weights / x / rope tables are pre-arranged on the host into the exact
    SBUF layout ([128 partitions, kt, cols]) so every load is one contiguous
    DMA with large descriptors.
  * Q/K are computed transposed ([head-dim, token]); per head the 128
    contraction rows are [qr-rot, qr-nr, qi-rot, qi-nr]; K2 = [-ki, kr] gives
    the imaginary logits with plain matmuls.
  * Scores are built [kv, q]; causality is exploited inside tiles by slicing
    all elementwise work (and matmuls where free>=256 keeps fp32r full rate)
    to the unmasked q-range; only diagonal tiles get a 128-wide affine_select.
  * sqrt/exp batched per head-pair so ACT table loads are amortized.
  * Softmax denominators accumulate in a dedicated PSUM row via ones-matmuls;
    one reciprocal per head-pair, broadcast across partitions with an
    indicator matmul; normalization multiplies read the AV sums directly from
    PSUM (no evacuation copies).
"""

import math

import numpy as np

B, S, DIM = 2, 1024, 1024
HEADS, HD, ROTD = 16, 64, 32
GH = 4  # heads per core
ODC = 256  # out-dim columns per core
NCORES = 8

_PAIRSWAP = [i ^ 1 for i in range(32)]

MM_DTYPE = "float32r"


def _register_magsq():
    """Register a fused custom DVE op: out = (in0^2 + in1^2) * imm2."""
    import numpy as np
    from concourse import dve_ops as DO
    from concourse.dve_spec import Spec, Src0, Src1, C2, sq, lower

    if "ANT_MAGSQ" in DO._SUB_OPCODE_FOR_NAME:
        return next(o for o in DO.OPS if o.name == "ANT_MAGSQ")
    spec = Spec(
        body=(sq(Src0) + sq(Src1)) * C2,
        reference=lambda in0, in1, s0, s1, imm2: (
            in0.astype(np.float32) ** 2 + in1.astype(np.float32) ** 2
        )
        * np.float32(imm2),
    )
    opcode = DO._CUSTOM_DVE_ROW_BASE + len(DO.OPS)
    DO._SUB_OPCODE_FOR_NAME["ANT_MAGSQ"] = opcode
    shas = {}
    for ver in ("v3", "v4"):
        try:
            s = DO.DveOpSpec(
                name="ANT_MAGSQ", opcode=opcode, uops=lower(spec, ver=ver), rd1_en=True
            )
            shas[ver] = s.sha(ver)
        except Exception:
            pass
    op = DO.DveOp("ANT_MAGSQ", spec, subdim=False, uops_sha=shas)
    DO.OPS.append(op)
    DO.CUSTOM_DVE_SPECS["ANT_MAGSQ"] = spec
    return op


def _build(gt: float, groups=None, mm_dtype=None):
    import concourse.mybir as mybir
    import concourse.tile as tile
    from concourse import bacc

    f32 = mybir.dt.float32
    bf16 = mybir.dt.bfloat16
    mdt = getattr(mybir.dt, mm_dtype or MM_DTYPE)
    AF = mybir.ActivationFunctionType
    magsq = _register_magsq()
    gt2 = float(gt) * float(gt)

    nc = bacc.Bacc("TRN2", target_bir_lowering=False, num_devices=NCORES)
    if groups is None:
        groups = [[0, 1, 2, 3], [4, 5, 6, 7]]

    # bf16 inputs halve the load-phase DMA (the kernel-start critical path)
    xr = nc.dram_tensor("xr_t", [128, 8, S], bf16, kind="ExternalInput")
    xi = nc.dram_tensor("xi_t", [128, 8, S], bf16, kind="ExternalInput")
    w = {
        nm: nc.dram_tensor(nm, [128, 8, 256], bf16, kind="ExternalInput")
        for nm in ["wqr", "wqi", "wkr", "wki", "wvr", "wvi"]
    }
    for nm in ["wor", "woi"]:
        w[nm] = nc.dram_tensor(nm, [128, 8, 256], bf16, kind="ExternalInput")
    bias_d = {
        nm: nc.dram_tensor(nm, [1, 256], mdt, kind="ExternalInput")
        for nm in ["bqr", "bqi", "bkr", "bki"]
    }
    # per-partition bias columns folded into PSUM-evacuation activations
    bnr_d = nc.dram_tensor("bnr", [128, 4], f32, kind="ExternalInput")
    boc_d = nc.dram_tensor("boc", [128, 4], f32, kind="ExternalInput")
    cosd = nc.dram_tensor("cosd", [128, S], f32, kind="ExternalInput")
    sind = nc.dram_tensor("sind", [128, S], f32, kind="ExternalInput")
    o_r = nc.dram_tensor("o_r", [ODC, S], bf16, kind="ExternalOutput")
    o_i = nc.dram_tensor("o_i", [ODC, S], bf16, kind="ExternalOutput")
    # one gather per chunk (more collectives serialize on the shared CC core);
    # rows grouped per head-pair: [hp0: h0r h1r h0i h1i | hp1: ...]
    agin_d = [
        nc.dram_tensor(f"agin{qc}", [512, 512], bf16, kind="Internal")
        for qc in range(2)
    ]
    agout_d = [
        nc.dram_tensor(f"agout{qc}", [4, 512, 512], bf16, kind="Internal")
        for qc in range(2)
    ]

    def mm(out, lhsT, rhs, start, stop):
        nc.tensor.matmul(out, lhsT=lhsT, rhs=rhs, start=start, stop=stop)

    # Rotate staging DMAs over the DMA-capable sequencers (SP/ACT HWDGE plus
    # the otherwise-idle gpsimd SWDGE).
    class DmaRotor:
        def __init__(self, engines):
            self.engines = engines
            self.i = 0

        def __call__(self):
            e = self.engines[self.i % len(self.engines)]
            self.i += 1
            return e

    rot = DmaRotor([nc.sync, nc.scalar, nc.sync, nc.gpsimd])
    rot2 = DmaRotor([nc.sync, nc.scalar])

    with tile.TileContext(nc) as tc:
        with (
            tc.tile_pool(name="consts", bufs=1) as consts,
            tc.tile_pool(name="qkpersist", bufs=1) as qkp,
            tc.tile_pool(name="vpersist", bufs=1) as vper,
            tc.tile_pool(name="wop", bufs=1) as wop,
            tc.tile_pool(name="ps", bufs=7, space="PSUM") as ps,
            tc.tile_pool(name="psd", bufs=1, space="PSUM") as psd,
        ):
            ones_t = consts.tile([1, 512], mdt, tag="ones")
            nc.vector.memset(ones_t.bitcast(f32), 1.0)
            # ones rows at partition bases 0 and 32 (matmul requires lhsT and
            # rhs to share their base partition) for the reciprocal broadcast
            ones_row = consts.tile([33, 64], f32, tag="onesr")
            nc.vector.memset(ones_row, 1.0)
            eps_t = consts.tile([128, 1], f32, tag="eps")
            nc.vector.memset(eps_t, 1e-6 * gt2)
            # bias tiles allocated here; DMAs issued after the first weight/x
            # loads so they don't delay the first matmul's inputs
            bias_sb = {
                nm: consts.tile([1, 256], mdt, tag=nm, name=f"bias_{nm}")
                for nm in ["bqr", "bqi", "bkr", "bki"]
            }
            bnr_sb = consts.tile([128, 4], f32, tag="bnr")
            boc_sb = consts.tile([128, 4], f32, tag="boc")

            # bf16 Q/K halves staging traffic, SBUF and stationary-load time;
            # logits stay fp32 in PSUM (measured rel err stays well in spec)
            Q = qkp.tile([128, GH, S], bf16, tag="Q")
            K1 = qkp.tile([128, GH, S], bf16, tag="K1")
            K2 = qkp.tile([128, GH, S], bf16, tag="K2")
            ovr = vper.tile([128, 8, 256], bf16, tag="ovr")
            ovi = vper.tile([128, 8, 256], bf16, tag="ovi")
            onescol_bf = consts.tile([128, 1], bf16, tag="onesc")
            nc.vector.memset(onescol_bf, 1.0)
            # wo tiles allocated here; their DMAs are issued after the
            # projection inputs (they're only needed at the very end)
            wo_sb = {}
            for ri, wname in ((0, "wor"), (1, "woi")):
                wo_sb[ri] = wop.tile([128, 8, 256], bf16, tag=wname, name=f"wo{ri}")

            # ---- projection phase (scoped pools so SBUF frees afterwards) ----
            with (
                tc.tile_pool(name="xp", bufs=1) as xp,
                tc.tile_pool(name="wqkp", bufs=1) as wqkp,
                tc.tile_pool(name="ropep", bufs=1) as ropep,
                tc.tile_pool(name="stage", bufs=2) as stage,
                tc.tile_pool(name="evp", bufs=2) as evp,
            ):
                # input DMAs in need-order: descriptors drain roughly in issue
                # order across the shared queues, so the first projection can
                # begin within a few microseconds
                w_sb = {}
                x_sb = {"r": [], "i": []}

                def load_w(nm):
                    t = wqkp.tile([128, 8, 256], bf16, tag=f"w{nm}")
                    rot2().dma_start(t, w[nm][:, :, :])
                    w_sb[nm] = t

                def load_x(key, src, kts):
                    for kt in kts:
                        t = xp.tile([128, S], bf16, tag=f"x{key}{kt}")
                        rot2().dma_start(t, src[:, kt, :])
                        x_sb[key].append(t)

                load_w("wqr")
                load_x("r", xr, range(8))
                for nm in ["bqr", "bqi", "bkr", "bki"]:
                    nc.sync.dma_start(bias_sb[nm], bias_d[nm][:, :])
                nc.scalar.dma_start(bnr_sb, bnr_d[:, :])
                nc.scalar.dma_start(boc_sb, boc_d[:, :])
                cos_sb = ropep.tile([128, S], f32, tag="cos")
                nc.sync.dma_start(cos_sb, cosd[:, :])
                sin_sb = ropep.tile([128, S], f32, tag="sin")
                nc.scalar.dma_start(sin_sb, sind[:, :])
                load_w("wqi")
                load_x("i", xi, range(8))
                for nm in ("wkr", "wki", "wvr", "wvi"):
                    load_w(nm)
                for ri, wname in ((0, "wor"), (1, "woi")):
                    rot2().dma_start(wo_sb[ri], w[wname][:, :, :])

                projs = [
                    ("qr", "r", "wqr", "bqr", [(0, 0)], [(0, 32)]),
                    ("qi", "i", "wqi", "bqi", [(0, 64)], [(0, 96)]),
                    ("kr", "r", "wkr", "bkr", [(1, 0), (2, 64)], [(1, 32), (2, 96)]),
                    ("ki", "i", "wki", "bki", [(1, 64)], [(1, 96)]),
                ]
                qk_tensors = {0: Q, 1: K1, 2: K2}

                for pidx, (pname, xkey, wname, bname, rot_tgts, nr_tgts) in enumerate(
                    projs
                ):
                    wt = w_sb[wname]
                    for mt in range(2):
                        psts = [
                            ps.tile([128, 512], f32, tag="ps", name=f"pq{c}")
                            for c in range(2)
                        ]
                        for kt in range(8):
                            for c in range(2):
                                mm(
                                    psts[c],
                                    wt[:, kt, mt * 128 : (mt + 1) * 128],
                                    x_sb[xkey][kt][:, c * 512 : (c + 1) * 512],
                                    start=(kt == 0),
                                    stop=(kt == 7 and mt == 1),
                                )
                        if mt == 0:
                            # rot-dim bias must be added before the rotation
                            for c in range(2):
                                mm(
                                    psts[c],
                                    bias_sb[bname][:, 0:128],
                                    ones_t,
                                    start=False,
                                    stop=True,
                                )
                        for c in range(2):
                            csl = slice(c * 512, (c + 1) * 512)
                            pst = psts[c]
                            if mt == 0:
                                shuf = stage.tile([128, 512], f32, tag="shuf")
                                nc.vector.stream_shuffle(shuf, pst, mask=_PAIRSWAP)
                                nc.vector.tensor_mul(shuf, shuf, sin_sb[:, csl])
                                t2f = stage.tile([128, 512], f32, tag="t2f")
                                nc.vector.tensor_mul(t2f, pst, cos_sb[:, csl])
                                t2 = stage.tile([128, 512], bf16, tag="t2")
                                nc.vector.tensor_add(t2, t2f, shuf)
                                src_t = t2
                            else:
                                # non-rot bias folds into the evacuation
                                evn = evp.tile([128, 512], bf16, tag="ev")
                                nc.scalar.activation(
                                    evn,
                                    pst,
                                    AF.Identity,
                                    bias=bnr_sb[:, pidx : pidx + 1],
                                )
                                src_t = evn
                            tgts = rot_tgts if mt == 0 else nr_tgts
                            for tid, row0 in tgts:
                                dst = qk_tensors[tid]
                                for h in range(GH):
                                    rot().dma_start(
                                        dst[row0 : row0 + 32, h, csl],
                                        src_t[h * 32 : (h + 1) * 32, :],
                                    )
                            if pname == "ki":
                                neg = evp.tile([128, 512], bf16, tag="neg")
                                nc.vector.tensor_scalar_mul(neg, src_t, -1.0)
                                row0 = 0 if mt == 0 else 32
                                for h in range(GH):
                                    rot().dma_start(
                                        K2[row0 : row0 + 32, h, csl],
                                        neg[h * 32 : (h + 1) * 32, :],
                                    )

                # V projections -> directly into persistent [tok, (h d)] tiles
                for xkey, wvname, ovt in (("r", "wvr", ovr), ("i", "wvi", ovi)):
                    wv_sb = w_sb[wvname]
                    for tt in range(8):
                        pv = ps.tile([128, 256], f32, tag="ps", name="pv")
                        for kt in range(8):
                            mm(
                                pv,
                                x_sb[xkey][kt][:, tt * 128 : (tt + 1) * 128],
                                wv_sb[:, kt, :],
                                start=(kt == 0),
                                stop=(kt == 7),
                            )
                        nc.scalar.copy(ovt[:, tt, :], pv)

            # ---- attention ----
            with (
                tc.tile_pool(name="sqp", bufs=40) as sqp,
                tc.tile_pool(name="etp", bufs=34) as etp,
                tc.tile_pool(name="ccp", bufs=4) as ccp,
                tc.tile_pool(name="onp", bufs=4) as onp,
                tc.tile_pool(name="smallp", bufs=2) as smallp,
                tc.tile_pool(name="lop", bufs=4) as lop,
                tc.tile_pool(name="oop", bufs=2) as oop,
            ):
                def score_phase(qc, act_copies):
                    """Scores + |z|^2 for all heads of chunk qc."""
                    nkv = (qc + 1) * 4
                    qcs = slice(qc * 512, (qc + 1) * 512)
                    sq_list = []
                    for h in range(GH):
                        qsl = Q[:, h, qcs]
                        for kvt in range(nkv):
                            loc = max(0, kvt * 128 - qc * 512)
                            ksl = slice(kvt * 128, (kvt + 1) * 128)
                            psr = ps.tile([128, 512], f32, tag="ps", name="psr")
                            mm(
                                psr[:, loc:],
                                K1[:, h, ksl],
                                qsl[:, loc:],
                                start=True,
                                stop=True,
                            )
                            psi = ps.tile([128, 512], f32, tag="ps", name="psi")
                            mm(
                                psi[:, loc:],
                                K2[:, h, ksl],
                                qsl[:, loc:],
                                start=True,
                                stop=True,
                            )
                            # GPSIMD can't read PSUM; the psi copy goes to ACT
                            # only when ACT isn't busy with another chunk's
                            # sqrt/exp batches.
                            c1 = ccp.tile([128, 512], f32, tag="c1")
                            if act_copies and (h + kvt) % 2 == 0:
                                nc.scalar.copy(c1[:, loc:], psi[:, loc:])
                            else:
                                nc.vector.tensor_copy(c1[:, loc:], psi[:, loc:])
                            sq = sqp.tile([128, 512], f32, tag="sq")
                            nc.vector._custom_dve(
                                magsq,
                                out=sq[:, loc:],
                                in0=psr[:, loc:],
                                in1=c1[:, loc:],
                                imm2=gt2,
                            )
                            sq_list.append((h, kvt, loc, sq))
                    return sq_list

                def act_phase(qc, sq_list):
                    """Batched sqrt/exp (amortized table loads) + causal masks."""
                    nkv = (qc + 1) * 4
                    for h, kvt, loc, sq in sq_list:
                        nc.scalar.activation(
                            sq[:, loc:], sq[:, loc:], AF.Sqrt, bias=eps_t
                        )
                    et_map = {}
                    for h, kvt, loc, sq in sq_list:
                        et = etp.tile([128, 512], bf16, tag="et")
                        nc.scalar.activation(
                            et[:, loc:], sq[:, loc:], AF.Exp, scale=1.0
                        )
                        et_map[(h, kvt)] = (et, loc)
                    for h in range(GH):
                        for kvt in range(nkv):
                            off = kvt - qc * 4
                            if off >= 0:
                                et, loc = et_map[(h, kvt)]
                                dlo = off * 128
                                nc.gpsimd.affine_select(
                                    out=et[:, dlo : dlo + 128],
                                    in_=et[:, dlo : dlo + 128],
                                    compare_op=mybir.AluOpType.is_ge,
                                    fill=0.0,
                                    base=0,
                                    channel_multiplier=-1,
                                    pattern=[[1, 128]],
                                )
                    return et_map

                def pair_phase(qc, et_map, hp):
                    nkv = (qc + 1) * 4
                    pair = (2 * hp, 2 * hp + 1)
                    # softmax denominators for the pair: PSUM rows 0 / 32
                    # (matmul out base partition must be 0, 32, or 64)
                    ddd = psd.tile([64, 512], f32, tag="ddd")
                    for hh, h in enumerate(pair):
                        r0 = hh * 32
                        for kvt in range(nkv):
                            et, loc = et_map[(h, kvt)]
                            mm(
                                ddd[r0 : r0 + 1, loc:],
                                onescol_bf,
                                et[:, loc:],
                                start=(kvt == 0),
                                stop=(kvt == nkv - 1),
                            )
                    # AV accumulation
                    avs = {}
                    for h in pair:
                        avr = ps.tile([64, 512], f32, tag="ps", name="avr")
                        avi = ps.tile([64, 512], f32, tag="ps", name="avi")
                        for kvt in range(nkv):
                            et, loc = et_map[(h, kvt)]
                            mm(
                                avr[:, loc:],
                                ovr[:, kvt, h * 64 : (h + 1) * 64],
                                et[:, loc:],
                                start=(kvt == 0),
                                stop=(kvt == nkv - 1),
                            )
                            mm(
                                avi[:, loc:],
                                ovi[:, kvt, h * 64 : (h + 1) * 64],
                                et[:, loc:],
                                start=(kvt == 0),
                                stop=(kvt == nkv - 1),
                            )
                        avs[h] = (avr, avi)
                    # normalize straight out of PSUM and stage for AllGather
                    # (approx reciprocal: denominators are sums of exp >= 1;
                    # garbage partitions 1-31/33-63 are never read)
                    lrec = smallp.tile([64, 512], f32, tag="lrec")
                    nc.vector.reciprocal_approx_fast(out=lrec, in_=ddd[:, :])
                    pb = ps.tile([128, 512], f32, tag="ps", name="pb")
                    mm(pb[0:64, :], ones_row[0:1, :], lrec[0:1, :],
                       start=True, stop=True)
                    mm(pb[64:128, :], ones_row[32:33, :], lrec[32:33, :],
                       start=True, stop=True)
                    pb_sb = smallp.tile([128, 512], f32, tag="pbs")
                    nc.scalar.copy(pb_sb, pb)
                    b0 = hp * 256
                    for hh, h in enumerate(pair):
                        avr, avi = avs[h]
                        bsl = slice(hh * 64, hh * 64 + 64)
                        onn = onp.tile([128, 512], bf16, tag="onn")
                        nc.vector.tensor_mul(onn[0:64, :], avr, pb_sb[bsl, :])
                        nc.vector.tensor_mul(onn[64:128, :], avi, pb_sb[bsl, :])
                        rot2().dma_start(
                            agin_d[qc][b0 + hh * 64 : b0 + (hh + 1) * 64, :],
                            onn[0:64, :],
                        )
                        rot2().dma_start(
                            agin_d[qc][b0 + 128 + hh * 64 : b0 + 128 + (hh + 1) * 64, :],
                            onn[64:128, :],
                        )

                def gather(qc):
                    nc.gpsimd.collective_compute(
                        "AllGather",
                        mybir.AluOpType.bypass,
                        replica_groups=groups,
                        ins=[agin_d[qc][:, :].opt()],
                        outs=[agout_d[qc][:, :, :].opt()],
                    )

                for qc in range(2):
                    sq = score_phase(qc, act_copies=True)
                    et = act_phase(qc, sq)
                    pair_phase(qc, et, 0)
                    pair_phase(qc, et, 1)
                    gather(qc)
                for qc in range(2):
                    qcs = slice(qc * 512, (qc + 1) * 512)
                    for ri, bname, odst in ((0, "bor", o_r), (1, "boi", o_i)):
                        pos = [
                            ps.tile([128, 512], f32, tag="ps", name=f"po{_i}")
                            for _i in range(2)
                        ]
                        for hp in range(2):
                            for g in range(4):
                                lt = lop.tile([128, 512], bf16, tag="lt")
                                nc.scalar.dma_start(
                                    lt,
                                    agout_d[qc][
                                        g,
                                        hp * 256 + ri * 128 : hp * 256
                                        + (ri + 1) * 128,
                                        :,
                                    ],
                                )
                                ht = g * 2 + hp
                                for odt in range(2):
                                    mm(
                                        pos[odt],
                                        wo_sb[ri][:, ht, odt * 128 : (odt + 1) * 128],
                                        lt,
                                        start=(hp == 0 and g == 0),
                                        stop=(hp == 1 and g == 3),
                                    )
                        for odt in range(2):
                            # output bias folds into the evacuation
                            oo = oop.tile([128, 512], bf16, tag="oo")
                            nc.scalar.activation(
                                oo,
                                pos[odt],
                                AF.Identity,
                                bias=boc_sb[:, ri * 2 + odt : ri * 2 + odt + 1],
                            )
                            nc.sync.dma_start(
                                odst[odt * 128 : (odt + 1) * 128, qcs], oo
                            )

    return nc


def _host_prep(inputs):
    """Fold ent/scale/bv on host; build per-core input maps (device layouts)."""
    import ml_dtypes

    bf16 = ml_dtypes.bfloat16
    f = lambda x: np.asarray(x, dtype=np.float32)
    real, imag = f(inputs["real"]), f(inputs["imag"])
    ent = np.asarray(inputs["ent"], np.float64)
    scale = 1.0 / math.sqrt(HD)

    def fold_w(W, do_ent, sc=1.0):
        W = np.asarray(W, np.float64).reshape(DIM, HEADS, HD)
        if do_ent:
            W = np.einsum("chd,hx->cxd", W, ent)
        return W * sc

    def fold_b(b, do_ent, sc=1.0):
        b = np.asarray(b, np.float64).reshape(HEADS, HD)
        if do_ent:
            b = np.einsum("hd,hx->xd", b, ent)
        return b * sc

    Wq_r = fold_w(inputs["Wq_r"], True, scale)
    Wq_i = fold_w(inputs["Wq_i"], True, scale)
    Wk_r = fold_w(inputs["Wk_r"], True)
    Wk_i = fold_w(inputs["Wk_i"], True)
    Wv_r = fold_w(inputs["Wv_r"], False)
    Wv_i = fold_w(inputs["Wv_i"], False)
    bq_r = fold_b(inputs["bq_r"], True, scale)
    bq_i = fold_b(inputs["bq_i"], True, scale)
    bk_r = fold_b(inputs["bk_r"], True)
    bk_i = fold_b(inputs["bk_i"], True)
    Wo_r = np.asarray(inputs["Wo_r"], np.float64)
    Wo_i = np.asarray(inputs["Wo_i"], np.float64)
    bo_r = np.asarray(inputs["bo_r"], np.float64) + np.asarray(
        inputs["bv_r"], np.float64
    ) @ Wo_r
    bo_i = np.asarray(inputs["bo_i"], np.float64) + np.asarray(
        inputs["bv_i"], np.float64
    ) @ Wo_i

    strength = float(np.asarray(inputs["strength"]).reshape(-1)[0])
    temp = float(np.asarray(inputs["temp"]).reshape(-1)[0])
    gt = (1.0 / (1.0 + math.exp(-strength))) / max(temp, 0.01)

    # rope tables in device layout: row h*32+d (d<32), freq j=d//2
    rot_freqs = np.asarray(inputs["rot_freqs"], np.float64)  # [16]
    pos = np.arange(S, dtype=np.float64)
    emb = pos[:, None] * rot_freqs[None, :]  # [S, 16]
    cos_t = np.cos(emb)
    sin_t = np.sin(emb)
    cosd = np.empty((128, S), np.float32)
    sind = np.empty((128, S), np.float32)
    for hh in range(4):
        for d in range(32):
            r = hh * 32 + d
            cosd[r] = cos_t[:, d // 2]
            sind[r] = (-sin_t if d % 2 == 0 else sin_t)[:, d // 2]

    def dev128(a2d, dtype=np.float32):
        # [1024, M] -> [128, 8, M] with row d = kt*128+kp at [kp, kt]
        m = a2d.shape[1]
        return np.ascontiguousarray(
            a2d.reshape(8, 128, m).transpose(1, 0, 2)
        ).astype(dtype)

    def qk_dev(Wf, bf_, g):
        hs = slice(g * GH, (g + 1) * GH)
        Wc, bc = Wf[:, hs, :], bf_[hs, :]
        wd = np.concatenate(
            [
                Wc[:, :, :ROTD].reshape(DIM, GH * ROTD),
                Wc[:, :, ROTD:].reshape(DIM, GH * ROTD),
            ],
            axis=1,
        )
        bd = np.concatenate(
            [bc[:, :ROTD].reshape(1, GH * ROTD), bc[:, ROTD:].reshape(1, GH * ROTD)],
            axis=1,
        )
        return wd, bd

    in_maps = []
    for core in range(NCORES):
        b, g = core // 4, core % 4
        hs = slice(g * GH, (g + 1) * GH)
        m = {
            "xr_t": dev128(np.ascontiguousarray(real[b].T), bf16),
            "xi_t": dev128(np.ascontiguousarray(imag[b].T), bf16),
            "cosd": cosd,
            "sind": sind,
            "wvr": dev128(Wv_r[:, hs, :].reshape(DIM, 256), bf16),
            "wvi": dev128(Wv_i[:, hs, :].reshape(DIM, 256), bf16),
            "wor": dev128(Wo_r[:, g * ODC : (g + 1) * ODC], bf16),
            "woi": dev128(Wo_i[:, g * ODC : (g + 1) * ODC], bf16),
            "boc": np.stack(
                [
                    bo[g * ODC + odt * 128 : g * ODC + (odt + 1) * 128]
                    for bo in (bo_r, bo_i)
                    for odt in range(2)
                ],
                axis=1,
            ).astype(np.float32),
        }
        bnr_cols = []
        for nm, Wf, bf_ in (
            ("qr", Wq_r, bq_r),
            ("qi", Wq_i, bq_i),
            ("kr", Wk_r, bk_r),
            ("ki", Wk_i, bk_i),
        ):
            wd, bd = qk_dev(Wf, bf_, g)
            m["w" + nm] = dev128(wd, bf16)
            m["b" + nm] = bd.astype(np.float32)
            bnr_cols.append(bd[0, 128:256])
        m["bnr"] = np.stack(bnr_cols, axis=1).astype(np.float32)
        in_maps.append(m)
    return in_maps, gt


def kernel(**inputs):
    from concourse import bass_utils

    in_maps, gt = _host_prep(inputs)
    nc = _build(gt)
    nc.finalize()
    res = bass_utils.run_bass_kernel_spmd(
        nc, in_maps, core_ids=list(range(NCORES))
    )
    out_r = np.empty((B, S, DIM), np.float32)
    out_i = np.empty((B, S, DIM), np.float32)
    for core in range(NCORES):
        b, g = core // 4, core % 4
        out_r[b, :, g * ODC : (g + 1) * ODC] = (
            res.results[core]["o_r"].astype(np.float32).T
        )
        out_i[b, :, g * ODC : (g + 1) * ODC] = (
            res.results[core]["o_i"].astype(np.float32).T
        )
    return np.stack([out_r, out_i], axis=0)
